# revision 2
# baseline (speedup 1.0000x reference)
"""Multi-head causal attention (B=2, S=2048, D=1024, H=16) on 8 Trainium2
NeuronCores.

Sharding: data-parallel over the 2 batches x tensor-parallel over 4 head
groups (4 heads each).  Core c handles batch c//4, heads [4*(c%4), 4*(c%4)+4).
Each core computes its Q/K/V projections from column shards of Wq/Wk/Wv,
runs causal attention for its heads, and applies its row shard of Wo,
producing a partial (D, S) output.  The host sums the 4 partials per batch
and adds the output bias.

On-core layout: activations are kept transposed (feature dim on SBUF
partitions, sequence on the free axis) so every matmul's operands are
already in the (K x M)/(K x N) form the PE array wants; the softmax
denominator comes free from an extra ones-row appended to V.

v2 structure (vs the first working version):
 - attention is emitted per (query-block, head-pair) with the two heads of
   a pair occupying disjoint 64-row groups of the PE array, so their score
   matmuls run concurrently (row tiling).
 - the softmax reciprocal runs on the Scalar engine as exp(-ln(x)) over a
   [4, 512] batch (both functions live in the natural_log_exp_and_others
   table set), replacing the 3.3 us/row DVE iterative divide.
 - x tiles are loaded as [128, 8, 1024] single DMAs (2 KiB lines, 2 MiB
   per transfer) instead of 8 separate 1 KiB-line transfers.
 - normalization + output projection are pipelined per query block instead
   of running after all attention.
"""

import sys

sys.path.insert(0, "/opt/trn_rl_repo")

import numpy as np

B, S, D, H = 2, 2048, 1024, 16
DK = D // H            # 64 head dim
NCORES = 8
NGROUPS = 4            # head groups (tensor parallel)
NH = H // NGROUPS      # 4 heads per core
DHL = NH * DK          # 256 local head dims per core
P = 128
DC = D // P            # 8 contraction chunks over D
HC = DHL // P          # 2 local head-dim chunks
SB = 512               # query block (matmul moving free size)
NSB = S // SB          # 4
SBH = 1024             # DMA/projection sequence chunk
NSBH = S // SBH        # 2
SCK = S // P           # 16 key chunks
G = 2                  # score chunks per exp group (2 PSUM banks)

_CACHE = {}


def _build_nc(causal):
    import concourse.bass as bass
    import concourse.bacc as bacc
    import concourse.mybir as mybir
    import concourse.tile as tile
    from contextlib import ExitStack

    f32 = mybir.dt.float32
    mmdt = mybir.dt.bfloat16
    Exp = mybir.ActivationFunctionType.Exp
    is_ge = mybir.AluOpType.is_ge

    nc = bacc.Bacc(None, target_bir_lowering=False, debug=False)

    xq_d = nc.dram_tensor("xq_t", [D, S], mmdt, kind="ExternalInput")
    xk_d = nc.dram_tensor("xk_t", [D, S], mmdt, kind="ExternalInput")
    xv_d = nc.dram_tensor("xv_t", [D, S], mmdt, kind="ExternalInput")
    # weights pre-arranged on host to the exact SBUF layouts
    wq_d = nc.dram_tensor("wq_a", [P, DC * DHL], mmdt, kind="ExternalInput")
    wk_d = nc.dram_tensor("wk_a", [P, DC * DHL], mmdt, kind="ExternalInput")
    wv_d = nc.dram_tensor("wv_a", [P, DC * DHL], mmdt, kind="ExternalInput")
    wo_d = nc.dram_tensor("wo_a", [P, HC * D], mmdt, kind="ExternalInput")
    bq_d = nc.dram_tensor("bq_a", [P, HC], f32, kind="ExternalInput")
    bk_d = nc.dram_tensor("bk_a", [P, HC], f32, kind="ExternalInput")
    bv_d = nc.dram_tensor("bv_a", [1, DHL], f32, kind="ExternalInput")
    out_d = nc.dram_tensor("out_t", [D, S], mmdt, kind="ExternalOutput")

    inv_sqrt_dk = 1.0 / float(np.sqrt(DK))

    with tile.TileContext(nc) as tc, ExitStack() as ctx:
        consts = ctx.enter_context(tc.tile_pool(name="consts", bufs=1))
        xpool = ctx.enter_context(tc.tile_pool(name="xpool", bufs=2))
        ex_pool = ctx.enter_context(tc.tile_pool(name="ex_pool", bufs=8))
        small = ctx.enter_context(tc.tile_pool(name="small", bufs=2))
        opool = ctx.enter_context(tc.tile_pool(name="opool", bufs=4))
        proj_ps = ctx.enter_context(
            tc.tile_pool(name="proj_ps", bufs=2, space="PSUM"))
        sc_ps = ctx.enter_context(
            tc.tile_pool(name="sc_ps", bufs=2, space="PSUM"))
        pv_ps = ctx.enter_context(
            tc.tile_pool(name="pv_ps", bufs=2, space="PSUM"))

        # --- resident tensors ---
        wq_sb = consts.tile([P, DC, DHL], mmdt)
        wk_sb = consts.tile([P, DC, DHL], mmdt)
        wv_sb = consts.tile([P, DC, DHL], mmdt)
        wo_sb = consts.tile([P, HC, D], mmdt)
        bq_sb = consts.tile([P, HC], f32)
        bk_sb = consts.tile([P, HC], f32)
        bv_row = consts.tile([1, DHL], f32)
        bv_bc = consts.tile([P, DHL], f32)
        q_sb = consts.tile([P, HC, S], mmdt)
        k_sb = consts.tile([P, HC, S], mmdt)
        v_aug = consts.tile([P, SCK, NH, DK + 1], mmdt)
        attn_sb = consts.tile([P, HC, S], mmdt)   # unnormalized PV
        attn_fin = consts.tile([P, HC, S], mmdt)  # normalized

        def emit_head_dmas(load_x0):
            # ordered so the first Q/K projection matmuls unblock earliest:
            # q/k weights + biases, then the x tiles, then V/O weights
            nc.sync.dma_start(
                wq_sb[:], wq_d[:].rearrange("p (c h) -> p c h", c=DC))
            nc.sync.dma_start(
                wk_sb[:], wk_d[:].rearrange("p (c h) -> p c h", c=DC))
            nc.sync.dma_start(bq_sb[:], bq_d[:])
            nc.sync.dma_start(bk_sb[:], bk_d[:])
            load_x0()
            nc.sync.dma_start(
                wv_sb[:], wv_d[:].rearrange("p (c h) -> p c h", c=DC))
            nc.sync.dma_start(bv_row[:], bv_d[:])
            nc.sync.dma_start(
                wo_sb[:], wo_d[:].rearrange("p (c o) -> p c o", c=HC))
            nc.gpsimd.partition_broadcast(bv_bc[:], bv_row[:])
            ones_f = consts.tile([P, SCK * NH], f32)
            nc.gpsimd.memset(ones_f[:], 1.0)
            nc.vector.tensor_copy(
                v_aug[:, :, :, DK],
                ones_f[:].rearrange("p (a b) -> p a b", a=SCK))
            nc.gpsimd.memset(ones_sb[:], 1.0)
            # warm up both GPSIMD ucode libraries (affine_select and
            # partition_broadcast) here, during the DMA-bound head —
            # otherwise the combined IRAM library load stalls every
            # engine ~7us at the first softmax normalize
            warm = small.tile([P, 64], f32, name="warm", tag="warm")
            nc.gpsimd.affine_select(
                warm[:], ones_sb[:, 0:64], pattern=[[1, 64]],
                compare_op=is_ge, fill=0.0, base=1, channel_multiplier=0)
            warm2 = small.tile([P, 64], f32, name="warm2", tag="warm")
            nc.gpsimd.partition_broadcast(warm2[:], ones_sb[0:1, 0:64])

        ones_sb = consts.tile([P, SB], f32)
        bv_bc_r = bv_bc[:].rearrange("p (h e) -> p h e", h=NH)

        def emit_attention(qb, fills):
            """Emit attention for query block qb, interleaving `fills`
            (projection / output-projection PSUM-group emitters) between
            attention groups so the PE always has metered fallback work
            while the Scalar engine paces the softmax."""
            def fill_one():
                if fills:
                    fills.pop(0)()
            n_chunks = (qb + 1) * (SB // P) if causal else SCK
            n_groups = n_chunks // G
            qs = slice(qb * SB, (qb + 1) * SB)
            # denominators for the 4 heads live at partitions 0/32/64/96
            # (partition bases must be 32-aligned)
            den4 = small.tile([P, SB], f32, name=f"den{qb}", tag="den")
            for p in range(2):  # head pair == hc index
                pv_t = [
                    pv_ps.tile([DK + 1, SB], f32, name=f"pv{qb}_{p}_{h2}",
                               tag="pv")
                    for h2 in range(2)
                ]
                for g in range(n_groups):
                    # diagonal chunks only need queries >= (tj - qb*4)*128:
                    # slice the moving operand (ragged causal trimming)
                    def chunk_qoff(tj):
                        return max(0, tj - qb * (SB // P)) * P if causal \
                            else 0
                    goff = chunk_qoff(g * G)
                    sc_ts = [
                        sc_ps.tile([P, G, SB], f32, name="sc", tag="sc")
                        for _ in range(2)
                    ]
                    # both heads' score matmuls adjacent: disjoint 64-row
                    # groups of the PE array -> run concurrently
                    for j2 in range(G):
                        tj = g * G + j2
                        qo = chunk_qoff(tj)
                        for h2 in range(2):
                            po = h2 * DK
                            nc.tensor.matmul(
                                sc_ts[h2][:, j2, qo:],
                                k_sb[po:po + DK, p, tj * P:(tj + 1) * P],
                                q_sb[po:po + DK, p,
                                     qb * SB + qo:(qb + 1) * SB],
                                start=True, stop=True)
                    ex_ts = []
                    for h2 in range(2):
                        ex = ex_pool.tile([P, G, SB], mmdt, name="ex",
                                          tag="ex")
                        nc.scalar.activation(
                            ex[:, :, goff:], sc_ts[h2][:, :, goff:], Exp,
                            bias=0.0, scale=inv_sqrt_dk)
                        if causal and g * G + G > qb * (SB // P):
                            d0 = g * G - qb * (SB // P)
                            # keep element when col - p - 128*(d0+j2) >= 0
                            nc.gpsimd.affine_select(
                                ex[:, :, goff:], ex[:, :, goff:],
                                pattern=[[-P, G], [1, SB - goff]],
                                compare_op=is_ge, fill=0.0,
                                base=goff - P * d0,
                                channel_multiplier=-1)
                        ex_ts.append(ex)
                    for h2 in range(2):
                        hl = 2 * p + h2
                        for j2 in range(G):
                            tj = g * G + j2
                            qo = chunk_qoff(tj)
                            nc.tensor.matmul(
                                pv_t[h2][:, qo:], v_aug[:, tj, hl, :],
                                ex_ts[h2][:, j2, qo:],
                                start=(g == 0 and j2 == 0),
                                stop=(g == n_groups - 1 and j2 == G - 1))
                    fill_one()
                # cross-partition-base writes must be tensor_tensor ops:
                # the BIR verifier requires TensorCopy in/out partitions
                # to match, but TT outputs may sit on other partitions.
                nc.vector.tensor_copy(
                    attn_sb[0:DK, p, qs], pv_t[0][0:DK, :])
                nc.vector.tensor_mul(
                    attn_sb[DK:P, p, qs], pv_t[1][0:DK, :],
                    ones_sb[0:DK, :])
                if qb == NSB - 1:
                    # last block: finalize per pair so only half the
                    # normalize chain sits in the kernel tail
                    den2 = small.tile([P, SB], f32, name=f"den{qb}_{p}",
                                      tag="den")
                    for h2 in range(2):
                        nc.vector.tensor_mul(
                            den2[32 * h2:32 * h2 + 1, :],
                            pv_t[h2][DK:DK + 1, :], ones_sb[DK:DK + 1, :])
                    recip2 = small.tile([P, SB], f32, name=f"recip{qb}_{p}",
                                        tag="recip4")
                    nc.vector.reciprocal(recip2[:], den2[:])
                    for h2 in range(2):
                        normalize_head(qb, qs, 2 * p + h2, recip2, 32 * h2)
                else:
                    for h2 in range(2):
                        hl = 2 * p + h2
                        nc.vector.tensor_mul(
                            den4[32 * hl:32 * hl + 1, :],
                            pv_t[h2][DK:DK + 1, :], ones_sb[DK:DK + 1, :])
                fill_one()
            if qb != NSB - 1:
                # batched softmax reciprocal: one full-tile DVE reciprocal
                # (cost is free-dim based, so [128, SB] costs the same as
                # one row; unused partitions hold garbage that's never read)
                recip4 = small.tile([P, SB], f32, name=f"recip{qb}",
                                    tag="recip4")
                nc.vector.reciprocal(recip4[:], den4[:])
                for hl in range(NH):
                    normalize_head(qb, qs, hl, recip4, 32 * hl)
                fill_one()
            while fills:
                fill_one()

        def normalize_head(qb, qs, hl, recip_t, row):
            p_, h2 = hl // 2, hl % 2
            po = h2 * DK
            # partition_broadcast only reads partition 0 correctly, so
            # stage the recip row at partition 0 first (DVE cross-base
            # copy is fine for 32-aligned bases)
            rN = small.tile([1, SB], mmdt, name=f"rN{qb}_{hl}",
                            tag="recipN")
            nc.vector.tensor_copy(rN[0:1, :], recip_t[row:row + 1, :])
            rbc = small.tile([P, SB], mmdt, name=f"rbc{qb}_{hl}",
                             tag="rbc")
            nc.gpsimd.partition_broadcast(rbc[:], rN[0:1, :])
            nc.vector.tensor_mul(
                attn_fin[po:po + DK, p_, qs],
                attn_sb[po:po + DK, p_, qs], rbc[po:po + DK, :])

        def make_proj_fills(sbi, xts):
            """PSUM-group closures for block sbi's projections: returns
            (qk, v) with qk = [q_hc0, q_hc1, k_hc0, k_hc1], v = [v0..v3]."""
            nn = sbi % (SBH // SB)
            ss = slice(sbi * SB, (sbi + 1) * SB)
            qk = []
            for name, w_sb, b_sb, t_sb in (
                    ("q", wq_sb, bq_sb, q_sb), ("k", wk_sb, bk_sb, k_sb)):
                for hc in range(HC):
                    def f(name=name, w_sb=w_sb, b_sb=b_sb, t_sb=t_sb, hc=hc):
                        xt = xts[name]
                        ps = proj_ps.tile([P, SB], f32, name="proj",
                                          tag="proj")
                        for dc in range(DC):
                            nc.tensor.matmul(
                                ps[:], w_sb[:, dc, hc * P:(hc + 1) * P],
                                xt[:, dc, nn * SB:(nn + 1) * SB],
                                start=(dc == 0), stop=(dc == DC - 1))
                        nc.vector.tensor_scalar_add(
                            t_sb[:, hc, ss], ps[:], b_sb[:, hc:hc + 1])
                    qk.append(f)
            v = []
            for scl4 in range(SB // P):
                def fv(scl4=scl4):
                    xt = xts["v"]
                    scl = nn * (SB // P) + scl4
                    sc_i = sbi * (SB // P) + scl4
                    ps = proj_ps.tile([P, DHL], f32, name="proj", tag="proj")
                    for dc in range(DC):
                        nc.tensor.matmul(
                            ps[:], xt[:, dc, scl * P:(scl + 1) * P],
                            wv_sb[:, dc, :],
                            start=(dc == 0), stop=(dc == DC - 1))
                    nc.vector.tensor_add(
                        v_aug[:, sc_i, :, 0:DK],
                        ps[:].rearrange("p (h e) -> p h e", h=NH), bv_bc_r)
                v.append(fv)
            return qk, v

        def make_oproj_fills(qb):
            """One closure per output-projection PSUM-group of block qb."""
            qs = slice(qb * SB, (qb + 1) * SB)
            fills = []
            for oc in range(DC):
                def f(oc=oc):
                    ps = proj_ps.tile([P, SB], f32, name="proj", tag="proj")
                    for hc2 in range(HC):
                        nc.tensor.matmul(
                            ps[:], wo_sb[:, hc2, oc * P:(oc + 1) * P],
                            attn_fin[:, hc2, qs],
                            start=(hc2 == 0), stop=(hc2 == HC - 1))
                    o_tile = opool.tile([P, SB], mmdt, name="ot", tag="ot")
                    nc.vector.tensor_copy(o_tile[:], ps[:])
                    nc.sync.dma_start(
                        out_d[:].rearrange("(c p) s -> p c s", p=P)
                        [:, oc, qs], o_tile[:])
                fills.append(f)
            return fills

        # --- pipeline: proj(0) first, then per query block qb run
        # attention(qb) with proj(qb+1) + O-proj(qb-1) interleaved as
        # metered PE fill work; O-proj(qb) for the last blocks drains at
        # the end of the following attention block / the kernel tail ---
        xts_by_sbh = {}

        def load_x(sbh):
            hs = slice(sbh * SBH, (sbh + 1) * SBH)
            xts = {}
            for name, x_d in (("q", xq_d), ("k", xk_d), ("v", xv_d)):
                xt = xpool.tile([P, DC, SBH], mmdt, name=f"x{name}{sbh}",
                                tag=f"x{name}")
                # split into dc-halves so projection matmuls can start
                # before the whole 2 MiB tile has landed
                src = x_d[:].rearrange("(c p) s -> p c s", p=P)[:, :, hs]
                half = DC // 2
                nc.sync.dma_start(xt[:, 0:half, :], src[:, 0:half, :])
                nc.sync.dma_start(xt[:, half:DC, :], src[:, half:DC, :])
                xts[name] = xt
            xts_by_sbh[sbh] = xts

        emit_head_dmas(lambda: load_x(0))
        # proj(0) fully up front (its DMAs gate everything anyway), then
        # hand-balanced fill lists: each attention block gets exactly as
        # many fills as it has pop points, ordered so every fill lands
        # before its first consumer and no big PE dump blocks the FIFO.
        qks, vs = {}, {}
        qks[0], vs[0] = make_proj_fills(0, xts_by_sbh[0])
        qks[1], vs[1] = make_proj_fills(1, xts_by_sbh[0])
        for f in qks[0] + vs[0]:
            f()
        op = {qb: make_oproj_fills(qb) for qb in range(NSB)}
        # qks[b] layout: [q_hc0, q_hc1, k_hc0, k_hc1]
        emit_attention(0, [qks[1][0], qks[1][2], vs[1][0], vs[1][1]])
        load_x(1)
        qks[2], vs[2] = make_proj_fills(2, xts_by_sbh[1])
        qks[3], vs[3] = make_proj_fills(3, xts_by_sbh[1])
        emit_attention(1, [vs[1][2], vs[1][3], qks[1][1], qks[1][3],
                           qks[2][0], qks[2][2], vs[2][0], vs[2][1]]
                       + op[0][0:3])
        emit_attention(2, [vs[2][2], vs[2][3], qks[2][1], qks[2][3],
                           qks[3][0], qks[3][2], vs[3][0], vs[3][1]]
                       + op[0][3:8] + op[1][0:2])
        emit_attention(3, [vs[3][2], vs[3][3], qks[3][1], qks[3][3]]
                       + op[1][2:8] + op[2][0:4])
        # reserved PE work for the kernel tail: runs during the last
        # block's softmax-normalize window and keeps the PE warm
        for f in op[2][4:8]:
            f()
        for f in op[NSB - 1]:
            f()

    nc.compile()
    return nc


def _get_nc(causal):
    key = ("causal" if causal else "dense")
    if key not in _CACHE:
        _CACHE[key] = _build_nc(causal)
    return _CACHE[key]


def _prep_core_inputs(Q, K, V, Wq, bq, Wk, bk, Wv, bv, Wo):
    """Build the 8 per-core input maps (all arrays C-contiguous)."""
    cc = np.ascontiguousarray
    in_maps = []
    for c in range(NCORES):
        b = c // NGROUPS
        g = c % NGROUPS
        hs, he = g * DHL, (g + 1) * DHL
        # weights pre-arranged to SBUF layout [128, DC, DHL] with d = dc*128+p
        wq_a = cc(Wq[hs:he, :].T.reshape(DC, P, DHL).transpose(1, 0, 2)
                  .reshape(P, DC * DHL))
        wk_a = cc(Wk[hs:he, :].T.reshape(DC, P, DHL).transpose(1, 0, 2)
                  .reshape(P, DC * DHL))
        wv_a = cc(Wv[hs:he, :].T.reshape(DC, P, DHL).transpose(1, 0, 2)
                  .reshape(P, DC * DHL))
        # Wo shard: lhsT layout [hd, dout] split to [128, HC, D], hd = hc*128+p
        wo_a = cc(Wo[:, hs:he].T.reshape(HC, P, D).transpose(1, 0, 2)
                  .reshape(P, HC * D))
        import ml_dtypes
        bf16 = ml_dtypes.bfloat16
        in_maps.append({
            "xq_t": cc(Q[b].T).astype(bf16), "xk_t": cc(K[b].T).astype(bf16),
            "xv_t": cc(V[b].T).astype(bf16),
            "wq_a": wq_a.astype(bf16), "wk_a": wk_a.astype(bf16),
            "wv_a": wv_a.astype(bf16), "wo_a": wo_a.astype(bf16),
            "bq_a": cc(bq[hs:he].reshape(HC, P).T),
            "bk_a": cc(bk[hs:he].reshape(HC, P).T),
            "bv_a": cc(bv[hs:he].reshape(1, DHL)),
        })
    return in_maps


def _classify_mask(mask):
    m = np.asarray(mask)
    if m.dtype != np.bool_:
        m = m.astype(bool)
    causal = np.tril(np.ones((S, S), dtype=bool))
    if all(np.array_equal(m[b, 0], causal) for b in range(m.shape[0])):
        return "causal"
    if m.all():
        return "dense"
    return "generic"


def _numpy_reference(Q, K, V, mask, Wq, bq, Wk, bk, Wv, bv, Wo, bo):
    """Plain numpy fallback for arbitrary masks."""
    out = np.empty((B, S, D), dtype=np.float32)
    for b in range(B):
        q = (Q[b] @ Wq.T + bq).reshape(S, H, DK).transpose(1, 0, 2)
        k = (K[b] @ Wk.T + bk).reshape(S, H, DK).transpose(1, 0, 2)
        v = (V[b] @ Wv.T + bv).reshape(S, H, DK).transpose(1, 0, 2)
        m = np.asarray(mask[b, 0], dtype=bool)
        acc = np.empty((H, S, DK), dtype=np.float32)
        for h in range(H):
            s = (q[h] @ k[h].T) / np.float32(np.sqrt(DK))
            s = np.where(m, s, np.float32(-1e9))
            s = s - s.max(axis=-1, keepdims=True)
            e = np.exp(s)
            p = e / e.sum(axis=-1, keepdims=True)
            acc[h] = p @ v[h]
        out[b] = acc.transpose(1, 0, 2).reshape(S, D) @ Wo.T + bo
    return out


def kernel(Q, K, V, mask, Wq, bq, Wk, bk, Wv, bv, Wo, bo,
           _profile=False, _trace_dir=None):
    from concourse.bass_utils import run_bass_kernel_spmd

    flavor = _classify_mask(mask)
    if flavor == "generic":
        return _numpy_reference(Q, K, V, mask, Wq, bq, Wk, bk, Wv, bv, Wo, bo)

    nc = _get_nc(flavor == "causal")
    in_maps = _prep_core_inputs(
        np.asarray(Q, np.float32), np.asarray(K, np.float32),
        np.asarray(V, np.float32), np.asarray(Wq, np.float32),
        np.asarray(bq, np.float32), np.asarray(Wk, np.float32),
        np.asarray(bk, np.float32), np.asarray(Wv, np.float32),
        np.asarray(bv, np.float32), np.asarray(Wo, np.float32))

    kwargs = {}
    if _profile:
        import concourse.bass_utils as _bu
        _bu.upload_artifacts = lambda d: d  # no cloud copy in this container
        kwargs = dict(trace=True, trace_cores=[0])
        if _trace_dir is not None:
            kwargs["tmpdir"] = _trace_dir
    res = run_bass_kernel_spmd(nc, in_maps, core_ids=list(range(NCORES)),
                               **kwargs)

    out = np.empty((B, S, D), dtype=np.float32)
    bo32 = np.asarray(bo, np.float32)
    for b in range(B):
        acc = res.results[b * NGROUPS]["out_t"].astype(np.float32)
        for g in range(1, NGROUPS):
            acc = acc + res.results[b * NGROUPS + g]["out_t"]
        out[b] = acc.T + bo32
    if _profile:
        kernel._last_exec_time_ns = res.exec_time_ns
        kernel._last_results = res
    return out



# revision 4
# speedup vs baseline: 1.1648x; 1.1648x over previous
"""Multi-head causal attention (B=2, S=2048, D=1024, H=16) on 8 Trainium2
NeuronCores.

Sharding: data-parallel over the 2 batches x tensor-parallel over 4 head
groups (4 heads each).  Core c handles batch c//4, heads [4*(c%4), 4*(c%4)+4).
Each core computes its Q/K/V projections from column shards of Wq/Wk/Wv,
runs causal attention for its heads, and applies its row shard of Wo,
producing a partial (D, S) output.  The host sums the 4 partials per batch
and adds the output bias.

On-core layout: activations are kept transposed (feature dim on SBUF
partitions, sequence on the free axis) so every matmul's operands are
already in the (K x M)/(K x N) form the PE array wants; the softmax
denominator comes free from an extra ones-row appended to V.

v3 structure (vs v2):
 - softmax reciprocal on the Scalar engine as exp(-ln(x)) (both funcs in
   the natural_log_exp_and_others table set) instead of the 4 us DVE
   iterative reciprocal; output directly bf16.
 - causal masking via precomputed multiplicative bf16 mask tiles applied
   with a ~0.2 us DVE tensor_mul instead of a ~1.4 us GPSIMD
   affine_select per diagonal group (the masks themselves are built once
   at startup with affine_select, which doubles as the ucode warmup).
 - last query block's normalize is fused (PSUM pv tile x broadcast recip
   -> attn_fin in one DVE op) to shorten the kernel tail.
 - head DMAs interleave weights with their x tensor (wq,bq,xq, wk,bk,xk,
   wv,bv,xv, wo) so the first projection unblocks earliest.
 - fills rebalanced so the PE has metered work across every attention
   block boundary.
"""

import sys

sys.path.insert(0, "/opt/trn_rl_repo")

import numpy as np

B, S, D, H = 2, 2048, 1024, 16
DK = D // H            # 64 head dim
NCORES = 8
NGROUPS = 4            # head groups (tensor parallel)
NH = H // NGROUPS      # 4 heads per core
DHL = NH * DK          # 256 local head dims per core
P = 128
DC = D // P            # 8 contraction chunks over D
HC = DHL // P          # 2 local head-dim chunks
SB = 512               # query block (matmul moving free size)
NSB = S // SB          # 4
SBH = 1024             # DMA/projection sequence chunk
NSBH = S // SBH        # 2
SCK = S // P           # 16 key chunks
G = 2                  # score chunks per exp group (2 PSUM banks)

_CACHE = {}


def _build_nc(causal):
    import concourse.bass as bass
    import concourse.bacc as bacc
    import concourse.mybir as mybir
    import concourse.tile as tile
    from contextlib import ExitStack

    f32 = mybir.dt.float32
    mmdt = mybir.dt.bfloat16
    Exp = mybir.ActivationFunctionType.Exp
    Ln = mybir.ActivationFunctionType.Ln
    is_ge = mybir.AluOpType.is_ge

    nc = bacc.Bacc(None, target_bir_lowering=False, debug=False)

    xq_d = nc.dram_tensor("xq_t", [D, S], mmdt, kind="ExternalInput")
    xk_d = nc.dram_tensor("xk_t", [D, S], mmdt, kind="ExternalInput")
    xv_d = nc.dram_tensor("xv_t", [D, S], mmdt, kind="ExternalInput")
    # weights pre-arranged on host to the exact SBUF layouts
    wq_d = nc.dram_tensor("wq_a", [P, DC * DHL], mmdt, kind="ExternalInput")
    wk_d = nc.dram_tensor("wk_a", [P, DC * DHL], mmdt, kind="ExternalInput")
    wv_d = nc.dram_tensor("wv_a", [P, DC * DHL], mmdt, kind="ExternalInput")
    wo_d = nc.dram_tensor("wo_a", [P, HC * D], mmdt, kind="ExternalInput")
    bq_d = nc.dram_tensor("bq_a", [P, HC], f32, kind="ExternalInput")
    bk_d = nc.dram_tensor("bk_a", [P, HC], f32, kind="ExternalInput")
    bv_d = nc.dram_tensor("bv_a", [1, DHL], f32, kind="ExternalInput")
    out_d = nc.dram_tensor("out_t", [D, S], mmdt, kind="ExternalOutput")

    inv_sqrt_dk = 1.0 / float(np.sqrt(DK))

    with tile.TileContext(nc) as tc, ExitStack() as ctx:
        consts = ctx.enter_context(tc.tile_pool(name="consts", bufs=1))
        xpool = ctx.enter_context(tc.tile_pool(name="xpool", bufs=2))
        ex_pool = ctx.enter_context(tc.tile_pool(name="ex_pool", bufs=8))
        small = ctx.enter_context(tc.tile_pool(name="small", bufs=2))
        opool = ctx.enter_context(tc.tile_pool(name="opool", bufs=4))
        proj_ps = ctx.enter_context(
            tc.tile_pool(name="proj_ps", bufs=2, space="PSUM"))
        sc_ps = ctx.enter_context(
            tc.tile_pool(name="sc_ps", bufs=2, space="PSUM"))
        pv_ps = ctx.enter_context(
            tc.tile_pool(name="pv_ps", bufs=2, space="PSUM"))

        # --- resident tensors ---
        wq_sb = consts.tile([P, DC, DHL], mmdt)
        wk_sb = consts.tile([P, DC, DHL], mmdt)
        wv_sb = consts.tile([P, DC, DHL], mmdt)
        wo_sb = consts.tile([P, HC, D], mmdt)
        bq_sb = consts.tile([P, HC], f32)
        bk_sb = consts.tile([P, HC], f32)
        bv_row = consts.tile([1, DHL], f32)
        bv_bc = consts.tile([P, DHL], f32)
        q_sb = consts.tile([P, HC, S], mmdt)
        k_sb = consts.tile([P, HC, S], mmdt)
        v_aug = consts.tile([P, SCK, NH, DK + 1], mmdt)
        attn_sb = consts.tile([P, HC, S], mmdt)   # unnormalized PV
        attn_fin = consts.tile([P, HC, S], mmdt)  # normalized
        ones_sb = consts.tile([P, SB], f32)
        # multiplicative causal masks for the two diagonal group offsets:
        # gm[d0][k, j, q] = 1 iff q >= k + 128*(d0+j)
        gm = {d0: consts.tile([P, G, SB], mmdt, name=f"gm{d0}")
              for d0 in (0, 2)}

        xts_by_sbh = {}

        def load_x_tensor(sbh, name, x_d):
            hs = slice(sbh * SBH, (sbh + 1) * SBH)
            xt = xpool.tile([P, DC, SBH], mmdt, name=f"x{name}{sbh}",
                            tag=f"x{name}")
            # split into dc-halves so projection matmuls can start
            # before the whole 2 MiB tile has landed
            src = x_d[:].rearrange("(c p) s -> p c s", p=P)[:, :, hs]
            half = DC // 2
            nc.sync.dma_start(xt[:, 0:half, :], src[:, 0:half, :])
            nc.sync.dma_start(xt[:, half:DC, :], src[:, half:DC, :])
            xts_by_sbh.setdefault(sbh, {})[name] = xt

        def emit_head_dmas():
            # ordered so the first Q projection matmuls unblock earliest:
            # each weight right before its x tensor
            nc.sync.dma_start(
                wq_sb[:], wq_d[:].rearrange("p (c h) -> p c h", c=DC))
            nc.sync.dma_start(bq_sb[:], bq_d[:])
            load_x_tensor(0, "q", xq_d)
            nc.sync.dma_start(
                wk_sb[:], wk_d[:].rearrange("p (c h) -> p c h", c=DC))
            nc.sync.dma_start(bk_sb[:], bk_d[:])
            load_x_tensor(0, "k", xk_d)
            nc.sync.dma_start(
                wv_sb[:], wv_d[:].rearrange("p (c h) -> p c h", c=DC))
            nc.sync.dma_start(bv_row[:], bv_d[:])
            load_x_tensor(0, "v", xv_d)
            nc.sync.dma_start(
                wo_sb[:], wo_d[:].rearrange("p (c o) -> p c o", c=HC))
            nc.gpsimd.partition_broadcast(bv_bc[:], bv_row[:])
            ones_f = consts.tile([P, SCK * NH], f32)
            nc.gpsimd.memset(ones_f[:], 1.0)
            nc.vector.tensor_copy(
                v_aug[:, :, :, DK],
                ones_f[:].rearrange("p (a b) -> p a b", a=SCK))
            nc.gpsimd.memset(ones_sb[:], 1.0)
            # build the causal mask tiles with affine_select on a ones
            # tile; this also warms the GPSIMD ucode libraries during the
            # DMA-bound head (combined IRAM library load would otherwise
            # stall every engine ~7us at first use)
            ones_bf = consts.tile([P, G, SB], mmdt)
            nc.gpsimd.memset(ones_bf[:], 1.0)
            for d0 in (0, 2):
                nc.gpsimd.affine_select(
                    gm[d0][:], ones_bf[:],
                    pattern=[[-P, G], [1, SB]],
                    compare_op=is_ge, fill=0.0,
                    base=-P * d0, channel_multiplier=-1)

        bv_bc_r = bv_bc[:].rearrange("p (h e) -> p h e", h=NH)

        def recip_act(dst, src, key):
            """dst = 1/src via exp(-ln(src)) on the Scalar engine."""
            ln_t = small.tile([P, SB], f32, name=f"ln{key}", tag="lnt")
            nc.scalar.activation(ln_t[:], src[:], Ln)
            nc.scalar.activation(dst[:], ln_t[:], Exp, bias=0.0, scale=-1.0)

        def emit_attention(qb, fills):
            """Emit attention for query block qb, interleaving `fills`
            (projection / output-projection PSUM-group emitters) between
            attention groups so the PE always has metered fallback work
            while the Scalar engine paces the softmax."""
            def fill_one():
                if fills:
                    fills.pop(0)()
            n_chunks = (qb + 1) * (SB // P) if causal else SCK
            n_groups = n_chunks // G
            qs = slice(qb * SB, (qb + 1) * SB)
            last = qb == NSB - 1
            den4 = None
            if not last:
                # denominators for the 4 heads live at partitions
                # 0/32/64/96 (partition bases must be 32-aligned)
                den4 = small.tile([P, SB], f32, name=f"den{qb}", tag="den")
            for p in range(2):  # head pair == hc index
                pv_t = [
                    pv_ps.tile([DK + 1, SB], f32, name=f"pv{qb}_{p}_{h2}",
                               tag="pv")
                    for h2 in range(2)
                ]
                for g in range(n_groups):
                    # diagonal chunks only need queries >= (tj - qb*4)*128:
                    # slice the moving operand (ragged causal trimming)
                    def chunk_qoff(tj):
                        return max(0, tj - qb * (SB // P)) * P if causal \
                            else 0
                    goff = chunk_qoff(g * G)
                    sc_ts = [
                        sc_ps.tile([P, G, SB], f32, name="sc", tag="sc")
                        for _ in range(2)
                    ]
                    # both heads' score matmuls adjacent: disjoint 64-row
                    # groups of the PE array -> run concurrently
                    for j2 in range(G):
                        tj = g * G + j2
                        qo = chunk_qoff(tj)
                        for h2 in range(2):
                            po = h2 * DK
                            nc.tensor.matmul(
                                sc_ts[h2][:, j2, qo:],
                                k_sb[po:po + DK, p, tj * P:(tj + 1) * P],
                                q_sb[po:po + DK, p,
                                     qb * SB + qo:(qb + 1) * SB],
                                start=True, stop=True)
                    ex_ts = []
                    for h2 in range(2):
                        ex = ex_pool.tile([P, G, SB], mmdt, name="ex",
                                          tag="ex")
                        nc.scalar.activation(
                            ex[:, :, goff:], sc_ts[h2][:, :, goff:], Exp,
                            bias=0.0, scale=inv_sqrt_dk)
                        if causal and g * G + G > qb * (SB // P):
                            d0 = g * G - qb * (SB // P)
                            nc.vector.tensor_mul(
                                ex[:, :, goff:], ex[:, :, goff:],
                                gm[d0][:, :, goff:])
                        ex_ts.append(ex)
                    for h2 in range(2):
                        hl = 2 * p + h2
                        for j2 in range(G):
                            tj = g * G + j2
                            qo = chunk_qoff(tj)
                            nc.tensor.matmul(
                                pv_t[h2][:, qo:], v_aug[:, tj, hl, :],
                                ex_ts[h2][:, j2, qo:],
                                start=(g == 0 and j2 == 0),
                                stop=(g == n_groups - 1 and j2 == G - 1))
                    fill_one()
                if last:
                    # last block: finalize per pair, normalize straight
                    # from the PSUM pv tiles to shorten the kernel tail
                    den2 = small.tile([P, SB], f32, name=f"den{qb}_{p}",
                                      tag="den")
                    for h2 in range(2):
                        nc.vector.tensor_mul(
                            den2[32 * h2:32 * h2 + 1, :],
                            pv_t[h2][DK:DK + 1, :], ones_sb[DK:DK + 1, :])
                    recip2 = small.tile([P, SB], mmdt, name=f"recip{qb}_{p}",
                                        tag="recip4")
                    recip_act(recip2, den2, f"{qb}_{p}")
                    for h2 in range(2):
                        hl = 2 * p + h2
                        po = h2 * DK
                        # partition_broadcast only reads partition 0
                        # correctly, so stage the recip row at partition 0
                        rN = small.tile([1, SB], mmdt, name=f"rN{qb}_{hl}",
                                        tag="recipN")
                        nc.vector.tensor_copy(
                            rN[0:1, :], recip2[32 * h2:32 * h2 + 1, :])
                        rbc = small.tile([P, SB], mmdt, name=f"rbc{qb}_{hl}",
                                         tag="rbc")
                        nc.gpsimd.partition_broadcast(rbc[:], rN[0:1, :])
                        nc.vector.tensor_mul(
                            attn_fin[po:po + DK, p, qs],
                            pv_t[h2][0:DK, :], rbc[po:po + DK, :])
                    fill_one()
                else:
                    # cross-partition-base writes must be tensor_tensor
                    # ops: the BIR verifier requires TensorCopy in/out
                    # partitions to match, but TT outputs may sit on other
                    # partitions.
                    nc.vector.tensor_copy(
                        attn_sb[0:DK, p, qs], pv_t[0][0:DK, :])
                    nc.vector.tensor_mul(
                        attn_sb[DK:P, p, qs], pv_t[1][0:DK, :],
                        ones_sb[0:DK, :])
                    for h2 in range(2):
                        hl = 2 * p + h2
                        nc.vector.tensor_mul(
                            den4[32 * hl:32 * hl + 1, :],
                            pv_t[h2][DK:DK + 1, :], ones_sb[DK:DK + 1, :])
                    fill_one()
            if not last:
                recip4 = small.tile([P, SB], mmdt, name=f"recip{qb}",
                                    tag="recip4")
                recip_act(recip4, den4, f"{qb}")
                for hl in range(NH):
                    normalize_head(qb, qs, hl, recip4, 32 * hl)
                fill_one()
            while fills:
                fill_one()

        def normalize_head(qb, qs, hl, recip_t, row):
            p_, h2 = hl // 2, hl % 2
            po = h2 * DK
            rN = small.tile([1, SB], mmdt, name=f"rN{qb}_{hl}",
                            tag="recipN")
            nc.vector.tensor_copy(rN[0:1, :], recip_t[row:row + 1, :])
            rbc = small.tile([P, SB], mmdt, name=f"rbc{qb}_{hl}",
                             tag="rbc")
            nc.gpsimd.partition_broadcast(rbc[:], rN[0:1, :])
            nc.vector.tensor_mul(
                attn_fin[po:po + DK, p_, qs],
                attn_sb[po:po + DK, p_, qs], rbc[po:po + DK, :])

        def make_proj_fills(sbi, xts):
            """PSUM-group closures for block sbi's projections: returns
            (qk, v) with qk = [q_hc0, q_hc1, k_hc0, k_hc1], v = [v0..v3]."""
            nn = sbi % (SBH // SB)
            ss = slice(sbi * SB, (sbi + 1) * SB)
            qk = []
            for name, w_sb, b_sb, t_sb in (
                    ("q", wq_sb, bq_sb, q_sb), ("k", wk_sb, bk_sb, k_sb)):
                for hc in range(HC):
                    def f(name=name, w_sb=w_sb, b_sb=b_sb, t_sb=t_sb, hc=hc):
                        xt = xts[name]
                        ps = proj_ps.tile([P, SB], f32, name="proj",
                                          tag="proj")
                        for dc in range(DC):
                            nc.tensor.matmul(
                                ps[:], w_sb[:, dc, hc * P:(hc + 1) * P],
                                xt[:, dc, nn * SB:(nn + 1) * SB],
                                start=(dc == 0), stop=(dc == DC - 1))
                        nc.vector.tensor_scalar_add(
                            t_sb[:, hc, ss], ps[:], b_sb[:, hc:hc + 1])
                    qk.append(f)
            v = []
            for scl4 in range(SB // P):
                def fv(scl4=scl4):
                    xt = xts["v"]
                    scl = nn * (SB // P) + scl4
                    sc_i = sbi * (SB // P) + scl4
                    ps = proj_ps.tile([P, DHL], f32, name="proj", tag="proj")
                    for dc in range(DC):
                        nc.tensor.matmul(
                            ps[:], xt[:, dc, scl * P:(scl + 1) * P],
                            wv_sb[:, dc, :],
                            start=(dc == 0), stop=(dc == DC - 1))
                    nc.vector.tensor_add(
                        v_aug[:, sc_i, :, 0:DK],
                        ps[:].rearrange("p (h e) -> p h e", h=NH), bv_bc_r)
                v.append(fv)
            return qk, v

        def make_oproj_fills(qb):
            """One closure per output-projection PSUM-group of block qb."""
            qs = slice(qb * SB, (qb + 1) * SB)
            fills = []
            for oc in range(DC):
                def f(oc=oc):
                    ps = proj_ps.tile([P, SB], f32, name="proj", tag="proj")
                    for hc2 in range(HC):
                        nc.tensor.matmul(
                            ps[:], wo_sb[:, hc2, oc * P:(oc + 1) * P],
                            attn_fin[:, hc2, qs],
                            start=(hc2 == 0), stop=(hc2 == HC - 1))
                    o_tile = opool.tile([P, SB], mmdt, name="ot", tag="ot")
                    nc.vector.tensor_copy(o_tile[:], ps[:])
                    nc.sync.dma_start(
                        out_d[:].rearrange("(c p) s -> p c s", p=P)
                        [:, oc, qs], o_tile[:])
                fills.append(f)
            return fills

        # --- pipeline: proj(0) first, then per query block qb run
        # attention(qb) with proj(qb+1) + O-proj(qb-1) interleaved as
        # metered PE fill work; O-proj for the last blocks drains at the
        # end of the following attention block / the kernel tail ---
        emit_head_dmas()
        qks, vs = {}, {}
        qks[0], vs[0] = make_proj_fills(0, xts_by_sbh[0])
        qks[1], vs[1] = make_proj_fills(1, xts_by_sbh[0])
        # proj(0) fully up front (its DMAs gate everything anyway), in
        # DMA-arrival order (xq, xk, xv)
        for f in [qks[0][0], qks[0][1], qks[0][2], qks[0][3]] + vs[0]:
            f()
        op = {qb: make_oproj_fills(qb) for qb in range(NSB)}
        # qks[b] layout: [q_hc0, q_hc1, k_hc0, k_hc1]
        # a(0): 7 pops
        emit_attention(0, [qks[1][0], qks[1][2], vs[1][0], vs[1][1],
                           vs[1][2], vs[1][3], qks[1][1]])
        load_x_tensor(1, "q", xq_d)
        load_x_tensor(1, "k", xk_d)
        load_x_tensor(1, "v", xv_d)
        qks[2], vs[2] = make_proj_fills(2, xts_by_sbh[1])
        qks[3], vs[3] = make_proj_fills(3, xts_by_sbh[1])
        # a(1): 11 pops
        emit_attention(1, [qks[1][3], qks[2][0], qks[2][2], qks[2][1],
                           qks[2][3], vs[2][0], vs[2][1], vs[2][2],
                           vs[2][3], op[0][0], op[0][1]])
        # a(2): 15 pops
        emit_attention(2, [qks[3][0], qks[3][2], qks[3][1], qks[3][3],
                           vs[3][0], vs[3][1], vs[3][2], vs[3][3]]
                       + op[0][2:8] + op[1][0:1])
        # a(3): 18 pops; op(2) placed late so its attn_fin(2) dependency
        # is ready by the time the PE FIFO reaches it
        emit_attention(3, op[1][1:8] + op[2][0:8])
        for f in op[NSB - 1]:
            f()

    nc.compile()
    return nc


def _get_nc(causal):
    key = ("causal" if causal else "dense")
    if key not in _CACHE:
        _CACHE[key] = _build_nc(causal)
    return _CACHE[key]


def _prep_core_inputs(Q, K, V, Wq, bq, Wk, bk, Wv, bv, Wo):
    """Build the 8 per-core input maps (all arrays C-contiguous)."""
    cc = np.ascontiguousarray
    in_maps = []
    for c in range(NCORES):
        b = c // NGROUPS
        g = c % NGROUPS
        hs, he = g * DHL, (g + 1) * DHL
        # weights pre-arranged to SBUF layout [128, DC, DHL] with d = dc*128+p
        wq_a = cc(Wq[hs:he, :].T.reshape(DC, P, DHL).transpose(1, 0, 2)
                  .reshape(P, DC * DHL))
        wk_a = cc(Wk[hs:he, :].T.reshape(DC, P, DHL).transpose(1, 0, 2)
                  .reshape(P, DC * DHL))
        wv_a = cc(Wv[hs:he, :].T.reshape(DC, P, DHL).transpose(1, 0, 2)
                  .reshape(P, DC * DHL))
        # Wo shard: lhsT layout [hd, dout] split to [128, HC, D], hd = hc*128+p
        wo_a = cc(Wo[:, hs:he].T.reshape(HC, P, D).transpose(1, 0, 2)
                  .reshape(P, HC * D))
        import ml_dtypes
        bf16 = ml_dtypes.bfloat16
        in_maps.append({
            "xq_t": cc(Q[b].T).astype(bf16), "xk_t": cc(K[b].T).astype(bf16),
            "xv_t": cc(V[b].T).astype(bf16),
            "wq_a": wq_a.astype(bf16), "wk_a": wk_a.astype(bf16),
            "wv_a": wv_a.astype(bf16), "wo_a": wo_a.astype(bf16),
            "bq_a": cc(bq[hs:he].reshape(HC, P).T),
            "bk_a": cc(bk[hs:he].reshape(HC, P).T),
            "bv_a": cc(bv[hs:he].reshape(1, DHL)),
        })
    return in_maps


def _classify_mask(mask):
    m = np.asarray(mask)
    if m.dtype != np.bool_:
        m = m.astype(bool)
    causal = np.tril(np.ones((S, S), dtype=bool))
    if all(np.array_equal(m[b, 0], causal) for b in range(m.shape[0])):
        return "causal"
    if m.all():
        return "dense"
    return "generic"


def _numpy_reference(Q, K, V, mask, Wq, bq, Wk, bk, Wv, bv, Wo, bo):
    """Plain numpy fallback for arbitrary masks."""
    out = np.empty((B, S, D), dtype=np.float32)
    for b in range(B):
        q = (Q[b] @ Wq.T + bq).reshape(S, H, DK).transpose(1, 0, 2)
        k = (K[b] @ Wk.T + bk).reshape(S, H, DK).transpose(1, 0, 2)
        v = (V[b] @ Wv.T + bv).reshape(S, H, DK).transpose(1, 0, 2)
        m = np.asarray(mask[b, 0], dtype=bool)
        acc = np.empty((H, S, DK), dtype=np.float32)
        for h in range(H):
            s = (q[h] @ k[h].T) / np.float32(np.sqrt(DK))
            s = np.where(m, s, np.float32(-1e9))
            s = s - s.max(axis=-1, keepdims=True)
            e = np.exp(s)
            p = e / e.sum(axis=-1, keepdims=True)
            acc[h] = p @ v[h]
        out[b] = acc.transpose(1, 0, 2).reshape(S, D) @ Wo.T + bo
    return out


def kernel(Q, K, V, mask, Wq, bq, Wk, bk, Wv, bv, Wo, bo,
           _profile=False, _trace_dir=None):
    from concourse.bass_utils import run_bass_kernel_spmd

    flavor = _classify_mask(mask)
    if flavor == "generic":
        return _numpy_reference(Q, K, V, mask, Wq, bq, Wk, bk, Wv, bv, Wo, bo)

    nc = _get_nc(flavor == "causal")
    in_maps = _prep_core_inputs(
        np.asarray(Q, np.float32), np.asarray(K, np.float32),
        np.asarray(V, np.float32), np.asarray(Wq, np.float32),
        np.asarray(bq, np.float32), np.asarray(Wk, np.float32),
        np.asarray(bk, np.float32), np.asarray(Wv, np.float32),
        np.asarray(bv, np.float32), np.asarray(Wo, np.float32))

    kwargs = {}
    if _profile:
        import concourse.bass_utils as _bu
        _bu.upload_artifacts = lambda d: d  # no cloud copy in this container
        kwargs = dict(trace=True, trace_cores=[0])
        if _trace_dir is not None:
            kwargs["tmpdir"] = _trace_dir
    res = run_bass_kernel_spmd(nc, in_maps, core_ids=list(range(NCORES)),
                               **kwargs)

    out = np.empty((B, S, D), dtype=np.float32)
    bo32 = np.asarray(bo, np.float32)
    for b in range(B):
        acc = res.results[b * NGROUPS]["out_t"].astype(np.float32)
        for g in range(1, NGROUPS):
            acc = acc + res.results[b * NGROUPS + g]["out_t"]
        out[b] = acc.T + bo32
    if _profile:
        kernel._last_exec_time_ns = res.exec_time_ns
        kernel._last_results = res
    return out


# revision 6
# speedup vs baseline: 1.2188x; 1.0464x over previous
"""Multi-head causal attention (B=2, S=2048, D=1024, H=16) on 8 Trainium2
NeuronCores.

Sharding: data-parallel over the 2 batches x tensor-parallel over 4 head
groups (4 heads each).  Core c handles batch c//4, heads [4*(c%4), 4*(c%4)+4).
Each core computes its Q/K/V projections from column shards of Wq/Wk/Wv,
runs causal attention for its heads, and applies its row shard of Wo,
producing a partial (D, S) output.  The host sums the 4 partials per batch
and adds the output bias.

On-core layout: activations are kept transposed (feature dim on SBUF
partitions, sequence on the free axis) so every matmul's operands are
already in the (K x M)/(K x N) form the PE array wants; the softmax
denominator comes free from an extra ones-row appended to V.

v3 structure (vs v2):
 - softmax reciprocal on the Scalar engine as exp(-ln(x)) (both funcs in
   the natural_log_exp_and_others table set) instead of the 4 us DVE
   iterative reciprocal; output directly bf16.
 - causal masking via precomputed multiplicative bf16 mask tiles applied
   with a ~0.2 us DVE tensor_mul instead of a ~1.4 us GPSIMD
   affine_select per diagonal group (the masks themselves are built once
   at startup with affine_select, which doubles as the ucode warmup).
 - last query block's normalize is fused (PSUM pv tile x broadcast recip
   -> attn_fin in one DVE op) to shorten the kernel tail.
 - head DMAs interleave weights with their x tensor (wq,bq,xq, wk,bk,xk,
   wv,bv,xv, wo) so the first projection unblocks earliest.
 - fills rebalanced so the PE has metered work across every attention
   block boundary.
"""

import sys

sys.path.insert(0, "/opt/trn_rl_repo")

import numpy as np

B, S, D, H = 2, 2048, 1024, 16
DK = D // H            # 64 head dim
NCORES = 8
NGROUPS = 4            # head groups (tensor parallel)
NH = H // NGROUPS      # 4 heads per core
DHL = NH * DK          # 256 local head dims per core
P = 128
DC = D // P            # 8 contraction chunks over D
HC = DHL // P          # 2 local head-dim chunks
SB = 512               # query block (matmul moving free size)
NSB = S // SB          # 4
SBH = 1024             # DMA/projection sequence chunk
NSBH = S // SBH        # 2
SCK = S // P           # 16 key chunks
G = 2                  # score chunks per exp group (2 PSUM banks)

_CACHE = {}


def _patch_act_tables():
    """Force Exp to resolve from the natural_log_exp_and_others table set
    (which also holds Ln) so the exp<->recip interleave doesn't thrash
    ACT_TABLE_LOADs (~1.3us each).  Only set membership used for set
    *choice* is filtered; names/order (the act_func_set_id space) are
    unchanged."""
    import functools
    import concourse.hw_specs as hw
    import concourse.bacc as bacc_mod
    import concourse.mybir as mybir

    orig = hw.get_activation_tables
    if getattr(orig, "_exp_pinned", False):
        return
    uncached = getattr(orig, "__wrapped__", orig)

    @functools.cache
    def patched(module_arch):
        t = dict(uncached(module_arch))
        Exp = mybir.ActivationFunctionType.Exp
        nl = "natural_log_exp_and_others"
        if nl in t and Exp in t[nl]:
            for name in list(t):
                if name != nl:
                    t[name] = t[name] - {Exp}
        return t

    patched._exp_pinned = True
    hw.get_activation_tables = patched
    bacc_mod.get_activation_tables = patched


def _build_nc(causal):
    import concourse.bass as bass
    import concourse.bacc as bacc
    import concourse.mybir as mybir
    import concourse.tile as tile
    from contextlib import ExitStack

    _patch_act_tables()

    f32 = mybir.dt.float32
    mmdt = mybir.dt.bfloat16
    Exp = mybir.ActivationFunctionType.Exp
    Ln = mybir.ActivationFunctionType.Ln
    is_ge = mybir.AluOpType.is_ge

    nc = bacc.Bacc(None, target_bir_lowering=False, debug=False)

    xq_d = nc.dram_tensor("xq_t", [D, S], mmdt, kind="ExternalInput")
    xk_d = nc.dram_tensor("xk_t", [D, S], mmdt, kind="ExternalInput")
    xv_d = nc.dram_tensor("xv_t", [D, S], mmdt, kind="ExternalInput")
    # weights pre-arranged on host to the exact SBUF layouts
    wq_d = nc.dram_tensor("wq_a", [P, DC * DHL], mmdt, kind="ExternalInput")
    wk_d = nc.dram_tensor("wk_a", [P, DC * DHL], mmdt, kind="ExternalInput")
    wv_d = nc.dram_tensor("wv_a", [P, DC * DHL], mmdt, kind="ExternalInput")
    wo_d = nc.dram_tensor("wo_a", [P, HC * D], mmdt, kind="ExternalInput")
    bq_d = nc.dram_tensor("bq_a", [P, HC], f32, kind="ExternalInput")
    bk_d = nc.dram_tensor("bk_a", [P, HC], f32, kind="ExternalInput")
    bv_d = nc.dram_tensor("bv_a", [1, DHL], f32, kind="ExternalInput")
    out_d = nc.dram_tensor("out_t", [D, S], mmdt, kind="ExternalOutput")

    inv_sqrt_dk = 1.0 / float(np.sqrt(DK))

    with tile.TileContext(nc) as tc, ExitStack() as ctx:
        consts = ctx.enter_context(tc.tile_pool(name="consts", bufs=1))
        xpool = ctx.enter_context(tc.tile_pool(name="xpool", bufs=2))
        ex_pool = ctx.enter_context(tc.tile_pool(name="ex_pool", bufs=8))
        small = ctx.enter_context(tc.tile_pool(name="small", bufs=2))
        opool = ctx.enter_context(tc.tile_pool(name="opool", bufs=4))
        proj_ps = ctx.enter_context(
            tc.tile_pool(name="proj_ps", bufs=2, space="PSUM"))
        sc_ps = ctx.enter_context(
            tc.tile_pool(name="sc_ps", bufs=2, space="PSUM"))
        pv_ps = ctx.enter_context(
            tc.tile_pool(name="pv_ps", bufs=2, space="PSUM"))

        # --- resident tensors ---
        wq_sb = consts.tile([P, DC, DHL], mmdt)
        wk_sb = consts.tile([P, DC, DHL], mmdt)
        wv_sb = consts.tile([P, DC, DHL], mmdt)
        wo_sb = consts.tile([P, HC, D], mmdt)
        bq_sb = consts.tile([P, HC], f32)
        bk_sb = consts.tile([P, HC], f32)
        bv_row = consts.tile([1, DHL], f32)
        bv_bc = consts.tile([P, DHL], f32)
        q_sb = consts.tile([P, HC, S], mmdt)
        k_sb = consts.tile([P, HC, S], mmdt)
        v_aug = consts.tile([P, SCK, NH, DK + 1], mmdt)
        attn_sb = consts.tile([P, HC, S], mmdt)   # unnormalized PV
        attn_fin = consts.tile([P, HC, S], mmdt)  # normalized
        ones_sb = consts.tile([P, SB], f32)
        # multiplicative causal masks for the two diagonal group offsets:
        # gm[d0][k, j, q] = 1 iff q >= k + 128*(d0+j)
        gm = {d0: consts.tile([P, G, SB], mmdt, name=f"gm{d0}")
              for d0 in (0, 2)}

        xts_by_sbh = {}

        def load_x_tensor(sbh, name, x_d):
            hs = slice(sbh * SBH, (sbh + 1) * SBH)
            xt = xpool.tile([P, DC, SBH], mmdt, name=f"x{name}{sbh}",
                            tag=f"x{name}")
            # split into dc-halves so projection matmuls can start
            # before the whole 2 MiB tile has landed
            src = x_d[:].rearrange("(c p) s -> p c s", p=P)[:, :, hs]
            half = DC // 2
            nc.sync.dma_start(xt[:, 0:half, :], src[:, 0:half, :])
            nc.sync.dma_start(xt[:, half:DC, :], src[:, half:DC, :])
            xts_by_sbh.setdefault(sbh, {})[name] = xt

        def emit_head_dmas():
            # ordered so the first Q projection matmuls unblock earliest:
            # each weight right before its x tensor
            nc.sync.dma_start(
                wq_sb[:], wq_d[:].rearrange("p (c h) -> p c h", c=DC))
            nc.sync.dma_start(bq_sb[:], bq_d[:])
            load_x_tensor(0, "q", xq_d)
            nc.sync.dma_start(
                wk_sb[:], wk_d[:].rearrange("p (c h) -> p c h", c=DC))
            nc.sync.dma_start(bk_sb[:], bk_d[:])
            load_x_tensor(0, "k", xk_d)
            nc.sync.dma_start(
                wv_sb[:], wv_d[:].rearrange("p (c h) -> p c h", c=DC))
            nc.sync.dma_start(bv_row[:], bv_d[:])
            load_x_tensor(0, "v", xv_d)
            nc.sync.dma_start(
                wo_sb[:], wo_d[:].rearrange("p (c o) -> p c o", c=HC))
            nc.gpsimd.partition_broadcast(bv_bc[:], bv_row[:])
            ones_f = consts.tile([P, SCK * NH], f32)
            nc.gpsimd.memset(ones_f[:], 1.0)
            nc.vector.tensor_copy(
                v_aug[:, :, :, DK],
                ones_f[:].rearrange("p (a b) -> p a b", a=SCK))
            nc.gpsimd.memset(ones_sb[:], 1.0)
            # build the causal mask tiles with affine_select on a ones
            # tile; this also warms the GPSIMD ucode libraries during the
            # DMA-bound head (combined IRAM library load would otherwise
            # stall every engine ~7us at first use)
            ones_bf = consts.tile([P, G, SB], mmdt)
            nc.gpsimd.memset(ones_bf[:], 1.0)
            for d0 in (0, 2):
                nc.gpsimd.affine_select(
                    gm[d0][:], ones_bf[:],
                    pattern=[[-P, G], [1, SB]],
                    compare_op=is_ge, fill=0.0,
                    base=-P * d0, channel_multiplier=-1)
            # warm the ACT function-table load (~2.7us) during the
            # DMA-bound head instead of stalling the first softmax exp
            act_warm = small.tile([1, 8], f32, name="act_warm", tag="warm")
            nc.scalar.activation(act_warm[:], ones_sb[0:1, 0:8], Exp)
            nc.scalar.activation(act_warm[:], act_warm[:], Ln)

        bv_bc_r = bv_bc[:].rearrange("p (h e) -> p h e", h=NH)

        def recip_act(dst, src, key):
            """dst = 1/src via exp(-ln(src)) on the Scalar engine."""
            ln_t = small.tile([P, SB], f32, name=f"ln{key}", tag="lnt")
            nc.scalar.activation(ln_t[:], src[:], Ln)
            nc.scalar.activation(dst[:], ln_t[:], Exp, bias=0.0, scale=-1.0)

        def emit_attention(qb, fills):
            """Emit attention for query block qb, interleaving `fills`
            (projection / output-projection PSUM-group emitters) between
            attention groups so the PE always has metered fallback work
            while the Scalar engine paces the softmax."""
            def fill_one():
                if fills:
                    fills.pop(0)()
            n_chunks = (qb + 1) * (SB // P) if causal else SCK
            n_groups = n_chunks // G
            qs = slice(qb * SB, (qb + 1) * SB)
            last = qb == NSB - 1
            den4 = None
            if not last:
                # denominators for the 4 heads live at partitions
                # 0/32/64/96 (partition bases must be 32-aligned)
                den4 = small.tile([P, SB], f32, name=f"den{qb}", tag="den")
            for p in range(2):  # head pair == hc index
                pv_t = [
                    pv_ps.tile([DK + 1, SB], f32, name=f"pv{qb}_{p}_{h2}",
                               tag="pv")
                    for h2 in range(2)
                ]
                for g in range(n_groups):
                    # diagonal chunks only need queries >= (tj - qb*4)*128:
                    # slice the moving operand (ragged causal trimming)
                    def chunk_qoff(tj):
                        return max(0, tj - qb * (SB // P)) * P if causal \
                            else 0
                    goff = chunk_qoff(g * G)
                    sc_ts = [
                        sc_ps.tile([P, G, SB], f32, name="sc", tag="sc")
                        for _ in range(2)
                    ]
                    # both heads' score matmuls adjacent: disjoint 64-row
                    # groups of the PE array -> run concurrently
                    for j2 in range(G):
                        tj = g * G + j2
                        qo = chunk_qoff(tj)
                        for h2 in range(2):
                            po = h2 * DK
                            nc.tensor.matmul(
                                sc_ts[h2][:, j2, qo:],
                                k_sb[po:po + DK, p, tj * P:(tj + 1) * P],
                                q_sb[po:po + DK, p,
                                     qb * SB + qo:(qb + 1) * SB],
                                start=True, stop=True)
                    ex_ts = []
                    for h2 in range(2):
                        ex = ex_pool.tile([P, G, SB], mmdt, name="ex",
                                          tag="ex")
                        nc.scalar.activation(
                            ex[:, :, goff:], sc_ts[h2][:, :, goff:], Exp,
                            bias=0.0, scale=inv_sqrt_dk)
                        if causal and g * G + G > qb * (SB // P):
                            d0 = g * G - qb * (SB // P)
                            nc.vector.tensor_mul(
                                ex[:, :, goff:], ex[:, :, goff:],
                                gm[d0][:, :, goff:])
                        ex_ts.append(ex)
                    for h2 in range(2):
                        hl = 2 * p + h2
                        for j2 in range(G):
                            tj = g * G + j2
                            qo = chunk_qoff(tj)
                            nc.tensor.matmul(
                                pv_t[h2][:, qo:], v_aug[:, tj, hl, :],
                                ex_ts[h2][:, j2, qo:],
                                start=(g == 0 and j2 == 0),
                                stop=(g == n_groups - 1 and j2 == G - 1))
                    fill_one()
                if last:
                    # last block: finalize per pair, normalize straight
                    # from the PSUM pv tiles to shorten the kernel tail
                    den2 = small.tile([P, SB], f32, name=f"den{qb}_{p}",
                                      tag="den")
                    for h2 in range(2):
                        nc.vector.tensor_mul(
                            den2[32 * h2:32 * h2 + 1, :],
                            pv_t[h2][DK:DK + 1, :], ones_sb[DK:DK + 1, :])
                    recip2 = small.tile([P, SB], mmdt, name=f"recip{qb}_{p}",
                                        tag="recip4")
                    recip_act(recip2, den2, f"{qb}_{p}")
                    for h2 in range(2):
                        hl = 2 * p + h2
                        po = h2 * DK
                        # partition_broadcast only reads partition 0
                        # correctly, so stage the recip row at partition 0
                        rN = small.tile([1, SB], mmdt, name=f"rN{qb}_{hl}",
                                        tag="recipN")
                        nc.vector.tensor_copy(
                            rN[0:1, :], recip2[32 * h2:32 * h2 + 1, :])
                        rbc = small.tile([P, SB], mmdt, name=f"rbc{qb}_{hl}",
                                         tag="rbc")
                        nc.gpsimd.partition_broadcast(rbc[:], rN[0:1, :])
                        nc.vector.tensor_mul(
                            attn_fin[po:po + DK, p, qs],
                            pv_t[h2][0:DK, :], rbc[po:po + DK, :])
                    fill_one()
                else:
                    # cross-partition-base writes must be tensor_tensor
                    # ops: the BIR verifier requires TensorCopy in/out
                    # partitions to match, but TT outputs may sit on other
                    # partitions.
                    nc.vector.tensor_copy(
                        attn_sb[0:DK, p, qs], pv_t[0][0:DK, :])
                    nc.vector.tensor_mul(
                        attn_sb[DK:P, p, qs], pv_t[1][0:DK, :],
                        ones_sb[0:DK, :])
                    for h2 in range(2):
                        hl = 2 * p + h2
                        nc.vector.tensor_mul(
                            den4[32 * hl:32 * hl + 1, :],
                            pv_t[h2][DK:DK + 1, :], ones_sb[DK:DK + 1, :])
                    fill_one()
            if not last:
                recip4 = small.tile([P, SB], mmdt, name=f"recip{qb}",
                                    tag="recip4")
                recip_act(recip4, den4, f"{qb}")
                for hl in range(NH):
                    normalize_head(qb, qs, hl, recip4, 32 * hl)
                fill_one()
            while fills:
                fill_one()

        def normalize_head(qb, qs, hl, recip_t, row):
            p_, h2 = hl // 2, hl % 2
            po = h2 * DK
            rN = small.tile([1, SB], mmdt, name=f"rN{qb}_{hl}",
                            tag="recipN")
            nc.vector.tensor_copy(rN[0:1, :], recip_t[row:row + 1, :])
            rbc = small.tile([P, SB], mmdt, name=f"rbc{qb}_{hl}",
                             tag="rbc")
            nc.gpsimd.partition_broadcast(rbc[:], rN[0:1, :])
            nc.vector.tensor_mul(
                attn_fin[po:po + DK, p_, qs],
                attn_sb[po:po + DK, p_, qs], rbc[po:po + DK, :])

        def make_proj_fills(sbi, xts):
            """PSUM-group closures for block sbi's projections: returns
            (qk, v) with qk = [q_hc0, q_hc1, k_hc0, k_hc1], v = [v0..v3]."""
            nn = sbi % (SBH // SB)
            ss = slice(sbi * SB, (sbi + 1) * SB)
            qk = []
            for name, w_sb, b_sb, t_sb in (
                    ("q", wq_sb, bq_sb, q_sb), ("k", wk_sb, bk_sb, k_sb)):
                for hc in range(HC):
                    def f(name=name, w_sb=w_sb, b_sb=b_sb, t_sb=t_sb, hc=hc):
                        xt = xts[name]
                        ps = proj_ps.tile([P, SB], f32, name="proj",
                                          tag="proj")
                        for dc in range(DC):
                            nc.tensor.matmul(
                                ps[:], w_sb[:, dc, hc * P:(hc + 1) * P],
                                xt[:, dc, nn * SB:(nn + 1) * SB],
                                start=(dc == 0), stop=(dc == DC - 1))
                        nc.vector.tensor_scalar_add(
                            t_sb[:, hc, ss], ps[:], b_sb[:, hc:hc + 1])
                    qk.append(f)
            v = []
            for scl4 in range(SB // P):
                def fv(scl4=scl4):
                    xt = xts["v"]
                    scl = nn * (SB // P) + scl4
                    sc_i = sbi * (SB // P) + scl4
                    ps = proj_ps.tile([P, DHL], f32, name="proj", tag="proj")
                    for dc in range(DC):
                        nc.tensor.matmul(
                            ps[:], xt[:, dc, scl * P:(scl + 1) * P],
                            wv_sb[:, dc, :],
                            start=(dc == 0), stop=(dc == DC - 1))
                    nc.vector.tensor_add(
                        v_aug[:, sc_i, :, 0:DK],
                        ps[:].rearrange("p (h e) -> p h e", h=NH), bv_bc_r)
                v.append(fv)
            return qk, v

        def make_oproj_fills(qb):
            """One closure per output-projection PSUM-group of block qb."""
            qs = slice(qb * SB, (qb + 1) * SB)
            fills = []
            for oc in range(DC):
                def f(oc=oc):
                    ps = proj_ps.tile([P, SB], f32, name="proj", tag="proj")
                    for hc2 in range(HC):
                        nc.tensor.matmul(
                            ps[:], wo_sb[:, hc2, oc * P:(oc + 1) * P],
                            attn_fin[:, hc2, qs],
                            start=(hc2 == 0), stop=(hc2 == HC - 1))
                    o_tile = opool.tile([P, SB], mmdt, name="ot", tag="ot")
                    nc.vector.tensor_copy(o_tile[:], ps[:])
                    nc.sync.dma_start(
                        out_d[:].rearrange("(c p) s -> p c s", p=P)
                        [:, oc, qs], o_tile[:])
                fills.append(f)
            return fills

        # --- pipeline: proj(0) first, then per query block qb run
        # attention(qb) with proj(qb+1) + O-proj(qb-1) interleaved as
        # metered PE fill work; O-proj for the last blocks drains at the
        # end of the following attention block / the kernel tail ---
        emit_head_dmas()
        qks, vs = {}, {}
        qks[0], vs[0] = make_proj_fills(0, xts_by_sbh[0])
        qks[1], vs[1] = make_proj_fills(1, xts_by_sbh[0])
        # proj(0) fully up front (its DMAs gate everything anyway), in
        # DMA-arrival order (xq, xk, xv)
        for f in [qks[0][0], qks[0][1], qks[0][2], qks[0][3]] + vs[0]:
            f()
        op = {qb: make_oproj_fills(qb) for qb in range(NSB)}
        # qks[b] layout: [q_hc0, q_hc1, k_hc0, k_hc1]
        # a(0): 7 pops
        emit_attention(0, [qks[1][0], qks[1][2], vs[1][0], vs[1][1],
                           vs[1][2], vs[1][3], qks[1][1]])
        load_x_tensor(1, "q", xq_d)
        load_x_tensor(1, "k", xk_d)
        load_x_tensor(1, "v", xv_d)
        qks[2], vs[2] = make_proj_fills(2, xts_by_sbh[1])
        qks[3], vs[3] = make_proj_fills(3, xts_by_sbh[1])
        # a(1): 11 pops
        emit_attention(1, [qks[1][3], qks[2][0], qks[2][2], qks[2][1],
                           qks[2][3], vs[2][0], vs[2][1], vs[2][2],
                           vs[2][3], op[0][0], op[0][1]])
        # a(2): 15 pops
        emit_attention(2, [qks[3][0], qks[3][2], qks[3][1], qks[3][3],
                           vs[3][0], vs[3][1], vs[3][2], vs[3][3]]
                       + op[0][2:8] + op[1][0:1])
        # a(3): 18 pops; op(2) placed late so its attn_fin(2) dependency
        # is ready by the time the PE FIFO reaches it
        emit_attention(3, op[1][1:8] + op[2][0:8])
        for f in op[NSB - 1]:
            f()

    nc.compile()
    return nc


def _get_nc(causal):
    key = ("causal" if causal else "dense")
    if key not in _CACHE:
        _CACHE[key] = _build_nc(causal)
    return _CACHE[key]


def _prep_core_inputs(Q, K, V, Wq, bq, Wk, bk, Wv, bv, Wo):
    """Build the 8 per-core input maps (all arrays C-contiguous)."""
    cc = np.ascontiguousarray
    in_maps = []
    for c in range(NCORES):
        b = c // NGROUPS
        g = c % NGROUPS
        hs, he = g * DHL, (g + 1) * DHL
        # weights pre-arranged to SBUF layout [128, DC, DHL] with d = dc*128+p
        wq_a = cc(Wq[hs:he, :].T.reshape(DC, P, DHL).transpose(1, 0, 2)
                  .reshape(P, DC * DHL))
        wk_a = cc(Wk[hs:he, :].T.reshape(DC, P, DHL).transpose(1, 0, 2)
                  .reshape(P, DC * DHL))
        wv_a = cc(Wv[hs:he, :].T.reshape(DC, P, DHL).transpose(1, 0, 2)
                  .reshape(P, DC * DHL))
        # Wo shard: lhsT layout [hd, dout] split to [128, HC, D], hd = hc*128+p
        wo_a = cc(Wo[:, hs:he].T.reshape(HC, P, D).transpose(1, 0, 2)
                  .reshape(P, HC * D))
        import ml_dtypes
        bf16 = ml_dtypes.bfloat16
        in_maps.append({
            "xq_t": cc(Q[b].T).astype(bf16), "xk_t": cc(K[b].T).astype(bf16),
            "xv_t": cc(V[b].T).astype(bf16),
            "wq_a": wq_a.astype(bf16), "wk_a": wk_a.astype(bf16),
            "wv_a": wv_a.astype(bf16), "wo_a": wo_a.astype(bf16),
            "bq_a": cc(bq[hs:he].reshape(HC, P).T),
            "bk_a": cc(bk[hs:he].reshape(HC, P).T),
            "bv_a": cc(bv[hs:he].reshape(1, DHL)),
        })
    return in_maps


def _classify_mask(mask):
    m = np.asarray(mask)
    if m.dtype != np.bool_:
        m = m.astype(bool)
    causal = np.tril(np.ones((S, S), dtype=bool))
    if all(np.array_equal(m[b, 0], causal) for b in range(m.shape[0])):
        return "causal"
    if m.all():
        return "dense"
    return "generic"


def _numpy_reference(Q, K, V, mask, Wq, bq, Wk, bk, Wv, bv, Wo, bo):
    """Plain numpy fallback for arbitrary masks."""
    out = np.empty((B, S, D), dtype=np.float32)
    for b in range(B):
        q = (Q[b] @ Wq.T + bq).reshape(S, H, DK).transpose(1, 0, 2)
        k = (K[b] @ Wk.T + bk).reshape(S, H, DK).transpose(1, 0, 2)
        v = (V[b] @ Wv.T + bv).reshape(S, H, DK).transpose(1, 0, 2)
        m = np.asarray(mask[b, 0], dtype=bool)
        acc = np.empty((H, S, DK), dtype=np.float32)
        for h in range(H):
            s = (q[h] @ k[h].T) / np.float32(np.sqrt(DK))
            s = np.where(m, s, np.float32(-1e9))
            s = s - s.max(axis=-1, keepdims=True)
            e = np.exp(s)
            p = e / e.sum(axis=-1, keepdims=True)
            acc[h] = p @ v[h]
        out[b] = acc.transpose(1, 0, 2).reshape(S, D) @ Wo.T + bo
    return out


def kernel(Q, K, V, mask, Wq, bq, Wk, bk, Wv, bv, Wo, bo,
           _profile=False, _trace_dir=None):
    from concourse.bass_utils import run_bass_kernel_spmd

    flavor = _classify_mask(mask)
    if flavor == "generic":
        return _numpy_reference(Q, K, V, mask, Wq, bq, Wk, bk, Wv, bv, Wo, bo)

    nc = _get_nc(flavor == "causal")
    in_maps = _prep_core_inputs(
        np.asarray(Q, np.float32), np.asarray(K, np.float32),
        np.asarray(V, np.float32), np.asarray(Wq, np.float32),
        np.asarray(bq, np.float32), np.asarray(Wk, np.float32),
        np.asarray(bk, np.float32), np.asarray(Wv, np.float32),
        np.asarray(bv, np.float32), np.asarray(Wo, np.float32))

    kwargs = {}
    if _profile:
        import concourse.bass_utils as _bu
        _bu.upload_artifacts = lambda d: d  # no cloud copy in this container
        kwargs = dict(trace=True, trace_cores=[0])
        if _trace_dir is not None:
            kwargs["tmpdir"] = _trace_dir
    res = run_bass_kernel_spmd(nc, in_maps, core_ids=list(range(NCORES)),
                               **kwargs)

    out = np.empty((B, S, D), dtype=np.float32)
    bo32 = np.asarray(bo, np.float32)
    for b in range(B):
        acc = res.results[b * NGROUPS]["out_t"].astype(np.float32)
        for g in range(1, NGROUPS):
            acc = acc + res.results[b * NGROUPS + g]["out_t"]
        out[b] = acc.T + bo32
    if _profile:
        kernel._last_exec_time_ns = res.exec_time_ns
        kernel._last_results = res
    return out
